# revision 1
# baseline (speedup 1.0000x reference)
"""Trainium2 Bass kernel for nn_BiAffineForward (bilinear relation scorer).

Strategy: data-parallel over the N=128 batch axis across 8 NeuronCores
(16 batches per core).  All index-derived gather/selection structures are
prepared host-side as small fp32 matrices so every device-side step is a
matmul / elementwise op:

  P1  span mean-pool      : per-batch matmul with span-weight matrix S
  T1  transpose           : PE transpose to feature-major layout
  P2  head/tail MLPs      : matmuls + relu (f1=140 split 128+12)
  P3  mention gather+mask : matmul with block-diagonal one-hot (384x160)
  P4  bilinear, stage 1   : per-r W_r-stationary matmul -> HWT[e,(n,i)]
  P5  bilinear, stage 2   : per-n tail-stationary matmul -> L[j,(r,i)]
  P6  masked logsumexp    : count-trick  out[n,r] = ln(sum_ij C*exp(L)) + b[r]

The logsumexp uses no max-subtraction: logits are O(1e-2) by construction
(weights scaled 0.02), and masked pairs contribute exactly 0 (their count
C is 0), which matches the reference's exp(-1e10)==0 behaviour in fp32.
"""

import sys

import numpy as np

sys.path.insert(0, "/opt/trn_rl_repo")

N, L, D = 128, 512, 768
M, H, T, HT = 24, 10, 10, 100
FF, R = 140, 97
NCORES = 8
NL = N // NCORES          # 16 batches per core
# Engine writes must start at 32-aligned partitions, so each batch's span
# rows / L rows live at offset 32*(n%4) inside a 128-partition tile.
SP = 512                  # padded span rows per core (16 batches x 32)
NH = NL * H               # 160 (n,i) columns per core
RE = R * FF               # 13580
NI = R * H                # 970 = L free size per batch (r-major, i-minor)
# r-halves for P4/P5 (keeps the HWT intermediate small enough for SBUF and
# each P5 psum within one 512-f32 bank); chunks stream the wde weights.
HALVES = ((0, 49), (49, 48))
CHUNKS = (((0, 25), (25, 24)), ((49, 24), (73, 24)))

_cache = {}


def _build():
    if "nc" in _cache:
        return _cache
    from contextlib import ExitStack

    import concourse.bacc as bacc
    import concourse.bass as bass
    import concourse.mybir as mybir
    import concourse.tile as tile
    from concourse.masks import make_identity

    f32 = mybir.dt.float32
    AF = mybir.ActivationFunctionType

    nc = bacc.Bacc(
        "TRN2",
        target_bir_lowering=False,
        debug=False,
        num_devices=NCORES,
        num_swdge_queues=4,
    )

    def din(name, shape, dt=f32):
        return nc.dram_tensor(name, list(shape), dt, kind="ExternalInput").ap()

    # span-row gather: 32 padded span slots x 8 rows per batch = 256 idxs
    sent = din("sent", (NL * L, D))
    gidx = din("gidx", (128, NL * 16), mybir.dt.int16)
    gpat = din("gpat", (128, NL, 2, M))
    w1h = din("w1h", (D, FF))
    w2h = din("w2h", (FF, FF))
    w1t = din("w1t", (D, FF))
    w2t = din("w2t", (FF, FF))
    gbh = din("gbh", (SP, NH))
    gbt = din("gbt", (SP, NH))
    wde = din("wde", (FF, RE), mybir.dt.bfloat16)   # [d, r*FF+e] = bili_W[r,d,e]
    cntd = din("cnt", (4, 128, NI))  # [(tile), (32*g+j), (r*10+i)] = C[n,i,j]
    spat = din("spat", (128, 4))     # partition-sum pattern
    bvec = din("bvec", (4, R))       # bili_b replicated on 4 rows
    outp = nc.dram_tensor("out", [NL, R], f32, kind="ExternalOutput").ap()

    with tile.TileContext(nc) as tc, ExitStack() as ctx:
        const = ctx.enter_context(tc.tile_pool(name="const", bufs=1))
        persist = ctx.enter_context(tc.tile_pool(name="persist", bufs=1))

        # ---- constants into SBUF
        w1h_sb = const.tile([128, 6, FF], f32)
        nc.sync.dma_start(w1h_sb[:], w1h.rearrange("(c p) f -> p c f", p=128))
        w1t_sb = const.tile([128, 6, FF], f32)
        nc.sync.dma_start(w1t_sb[:], w1t.rearrange("(c p) f -> p c f", p=128))
        w2h_sb = const.tile([128, 2, FF], f32)
        nc.sync.dma_start(w2h_sb[:, 0, :], w2h[0:128, :])
        nc.sync.dma_start(w2h_sb[0:12, 1, :], w2h[128:FF, :])
        w2t_sb = const.tile([128, 2, FF], f32)
        nc.sync.dma_start(w2t_sb[:, 0, :], w2t[0:128, :])
        nc.sync.dma_start(w2t_sb[0:12, 1, :], w2t[128:FF, :])
        gbh_sb = const.tile([128, 4, NH], f32)
        nc.sync.dma_start(gbh_sb[:], gbh.rearrange("(c p) g -> p c g", p=128))
        gbt_sb = const.tile([128, 4, NH], f32)
        nc.sync.dma_start(gbt_sb[:], gbt.rearrange("(c p) g -> p c g", p=128))
        cnt_sb = const.tile([128, 4, NI], f32)
        nc.sync.dma_start(cnt_sb[:], cntd.rearrange("t p f -> p t f"))
        spat_sb = const.tile([128, 4], f32)
        nc.sync.dma_start(spat_sb[:], spat[:, :])
        bvec_sb = const.tile([4, R], f32)
        nc.sync.dma_start(bvec_sb[:], bvec[:, :])
        ident = const.tile([128, 128], f32)
        make_identity(nc, ident)
        gidx_sb = const.tile([128, NL * 16], mybir.dt.int16)
        nc.sync.dma_start(gidx_sb[:], gidx[:, :])
        gpat_sb = const.tile([128, NL, 2, M], f32)
        nc.sync.dma_start(gpat_sb[:], gpat[:, :, :, :])

        # ---- persistent intermediates (head side bf16: P4 operands)
        bf16 = mybir.dt.bfloat16
        hg0 = persist.tile([128, NH], bf16)
        hg1 = persist.tile([12, NH], bf16)
        tg0 = persist.tile([128, NH], f32)
        tg1 = persist.tile([12, NH], f32)

        # =========================== Phase A ===========================
        with tc.tile_pool(name="pa_sb", bufs=1) as pa, \
             tc.tile_pool(name="pa_str", bufs=4) as pstr, \
             tc.tile_pool(name="pa_ps", bufs=1, space="PSUM") as pps, \
             tc.tile_pool(name="pa_ps2", bufs=1, space="PSUM") as pps2:

            spans_A = pa.tile([128, 4, D], f32)
            nc.gpsimd.memset(spans_A[:], 0.0)
            for c in range(NL // 2):      # gather 2 batches (512 rows) per chunk
                gt = pstr.tile([128, 4, D], f32, tag="gt")
                nc.gpsimd.dma_gather(
                    out_ap=gt[:, :, :],
                    in_ap=sent[:, :],
                    idxs_ap=gidx_sb[:, c * 32:(c + 1) * 32],
                    num_idxs=512,
                    num_idxs_reg=512,
                    elem_size=D,
                    queue_num=c % 4,
                )
                for b in range(2):
                    n = 2 * c + b
                    ps_sp = pps.tile([M, D], f32, tag="ps_sp", bufs=2)
                    for off, wdt in ((0, 512), (512, 256)):
                        for te in range(2):
                            nc.tensor.matmul(
                                ps_sp[:, off:off + wdt],
                                lhsT=gpat_sb[:, n, te, :],
                                rhs=gt[:, 2 * b + te, off:off + wdt],
                                start=(te == 0),
                                stop=(te == 1),
                            )
                    # batch n's 24 span rows -> rows 32*(n%4).. of tile n//4
                    p0 = 32 * (n % 4)
                    nc.scalar.copy(spans_A[p0:p0 + M, n // 4, :], ps_sp[:])

            # T1: transpose to spansT (d on partitions)
            spansT = pa.tile([128, 6, SP], f32)
            for t3 in range(4):
                for dc in range(6):
                    ps_t = pps2.tile([128, 128], f32, tag="ps_t")
                    nc.tensor.transpose(
                        ps_t[:], spans_A[:, t3, dc * 128:(dc + 1) * 128], ident[:]
                    )
                    nc.vector.tensor_copy(
                        spansT[:, dc, t3 * 128:(t3 + 1) * 128], ps_t[:]
                    )

            # P2 + P3 for head and tail
            for w1sb, w2sb, gbsb, g0, g1 in (
                (w1h_sb, w2h_sb, gbh_sb, hg0, hg1),
                (w1t_sb, w2t_sb, gbt_sb, tg0, tg1),
            ):
                a0 = pa.tile([128, SP], f32, tag="a0")
                a1 = pa.tile([12, SP], f32, tag="a1")
                for mc, (mo, mw) in enumerate(((0, 128), (128, 12))):
                    ps_a = pps.tile([128, SP], f32, tag="ps_a")
                    for kc in range(6):
                        nc.tensor.matmul(
                            ps_a[0:mw, :],
                            lhsT=w1sb[:, kc, mo:mo + mw],
                            rhs=spansT[:, kc, :],
                            start=(kc == 0),
                            stop=(kc == 5),
                        )
                    tgt = a0 if mc == 0 else a1
                    nc.scalar.activation(tgt[0:mw, :], ps_a[0:mw, :], AF.Relu)
                b2 = pa.tile([128, 4, FF], f32, tag="b2")
                for mc3 in range(4):
                    ps_b = pps2.tile([128, FF], f32, tag="ps_b")
                    sl = slice(mc3 * 128, (mc3 + 1) * 128)
                    nc.tensor.matmul(
                        ps_b[:], lhsT=a0[:, sl], rhs=w2sb[:, 0, :],
                        start=True, stop=False,
                    )
                    nc.tensor.matmul(
                        ps_b[:], lhsT=a1[:, sl], rhs=w2sb[0:12, 1, :],
                        start=False, stop=True,
                    )
                    nc.vector.tensor_copy(b2[:, mc3, :], ps_b[:])
                for mc, (mo, mw) in enumerate(((0, 128), (128, 12))):
                    ps_g = pps.tile([128, NH], f32, tag="ps_g")
                    for kc3 in range(4):
                        nc.tensor.matmul(
                            ps_g[0:mw, :],
                            lhsT=b2[:, kc3, mo:mo + mw],
                            rhs=gbsb[:, kc3, :],
                            start=(kc3 == 0),
                            stop=(kc3 == 3),
                        )
                    tgt = g0 if mc == 0 else g1
                    nc.vector.tensor_copy(tgt[0:mw, :], ps_g[0:mw, :])

        # =========================== Phase B ===========================
        with tc.tile_pool(name="hwtp", bufs=1) as hwtp, \
             tc.tile_pool(name="wdep", bufs=2) as wdep, \
             tc.tile_pool(name="ltp", bufs=1) as ltp, \
             tc.tile_pool(name="p6", bufs=2) as p6, \
             tc.tile_pool(name="bpsA", bufs=3, space="PSUM") as bpsA, \
             tc.tile_pool(name="bpsB", bufs=2, space="PSUM") as bpsB, \
             tc.tile_pool(name="bps5", bufs=2, space="PSUM") as bps5, \
             tc.tile_pool(name="bps6", bufs=1, space="PSUM") as bps6:
            lts = [ltp.tile([128, NI], f32, name=f"lt{t}") for t in range(4)]
            for lt in lts:
                nc.gpsimd.memset(lt[:], 0.0)
            for (rh0, rhw), chunks in zip(HALVES, CHUNKS):
                FR = rhw * H                     # 490 | 480 L-cols per batch
                # hwt col layout within this half: n*FR + rl*10 + i
                hwt0h = hwtp.tile([128, NL * FR], f32, tag="hwt0h")
                hwremh = hwtp.tile([12, NL * FR], f32, tag="hwremh")
                hw_v = hwt0h.rearrange("p (n r i) -> p n r i", n=NL, r=rhw)
                hwr_v = hwremh.rearrange("p (n r i) -> p n r i", n=NL, r=rhw)
                for c0, cw in chunks:
                    wt = wdep.tile([128, 25 * FF], bf16, tag="wt")
                    nc.sync.dma_start(
                        wt[:, 0:cw * FF], wde[0:128, c0 * FF:(c0 + cw) * FF]
                    )
                    wtr = wdep.tile([12, 25 * FF], bf16, tag="wtr")
                    nc.sync.dma_start(
                        wtr[:, 0:cw * FF], wde[128:FF, c0 * FF:(c0 + cw) * FF]
                    )
                    for rl in range(cw):
                        rloc = c0 - rh0 + rl     # r-index within the half
                        psA = bpsA.tile([128, NH], f32, tag="psA")
                        nc.tensor.matmul(
                            psA[:], lhsT=wt[:, rl * FF:rl * FF + 128],
                            rhs=hg0[:], start=True, stop=False,
                        )
                        nc.tensor.matmul(
                            psA[:], lhsT=wtr[:, rl * FF:rl * FF + 128],
                            rhs=hg1[:], start=False, stop=True,
                        )
                        psB = bpsB.tile([12, NH], f32, tag="psB")
                        nc.tensor.matmul(
                            psB[:], lhsT=wt[:, rl * FF + 128:rl * FF + FF],
                            rhs=hg0[:], start=True, stop=False,
                        )
                        nc.tensor.matmul(
                            psB[:], lhsT=wtr[:, rl * FF + 128:rl * FF + FF],
                            rhs=hg1[:], start=False, stop=True,
                        )
                        nc.vector.tensor_copy(
                            hw_v[:, :, rloc, :],
                            psA.rearrange("p (n i) -> p n i", n=NL),
                        )
                        nc.vector.tensor_copy(
                            hwr_v[:, :, rloc, :],
                            psB.rearrange("p (n i) -> p n i", n=NL),
                        )

                # P5 for this half
                for n in range(NL):
                    ps5 = bps5.tile([10, 490], f32, tag="ps5")
                    nc.tensor.matmul(
                        ps5[:, 0:FR], lhsT=tg0[:, n * 10:(n + 1) * 10],
                        rhs=hwt0h[:, n * FR:(n + 1) * FR],
                        start=True, stop=False,
                    )
                    nc.tensor.matmul(
                        ps5[:, 0:FR], lhsT=tg1[:, n * 10:(n + 1) * 10],
                        rhs=hwremh[:, n * FR:(n + 1) * FR],
                        start=False, stop=True,
                    )
                    lt = lts[n // 4]
                    po = 32 * (n % 4)
                    nc.vector.tensor_copy(
                        lt[po:po + 10, rh0 * H:rh0 * H + FR], ps5[:, 0:FR]
                    )

            # ---- P6: count-trick masked logsumexp
            for t2 in range(4):
                lt = lts[t2]
                et = p6.tile([128, NI], f32, tag="et")
                nc.scalar.activation(et[:], lt[:], AF.Exp)
                mt = p6.tile([128, NI], f32, tag="mt")
                nc.vector.tensor_mul(mt[:], et[:], cnt_sb[:, t2, :])
                s1 = p6.tile([128, R], f32, tag="s1")
                nc.vector.tensor_reduce(
                    s1[:],
                    mt.rearrange("p (r i) -> p r i", r=R),
                    axis=mybir.AxisListType.X,
                    op=mybir.AluOpType.add,
                )
                ps6 = bps6.tile([4, R], f32, tag="ps6")
                nc.tensor.matmul(
                    ps6[:], lhsT=spat_sb[:], rhs=s1[:], start=True, stop=True
                )
                lg = p6.tile([4, R], f32, tag="lg")
                nc.scalar.activation(lg[:], ps6[:], AF.Ln)
                res = p6.tile([4, R], f32, tag="res")
                nc.vector.tensor_add(res[:], lg[:], bvec_sb[:])
                nc.sync.dma_start(outp[t2 * 4:(t2 + 1) * 4, :], res[:])

    nc.compile()
    _cache["nc"] = nc
    return _cache


def _host_prep(inputs):
    """Shard + build index-derived matrices. Returns in_maps (one per core)."""
    sent_f = np.ascontiguousarray(np.asarray(inputs["sentence_repr"], np.float32))
    spans = np.asarray(inputs["entity_span_indices"]).astype(np.int64)
    hidx = np.asarray(inputs["head_mentions_indices"]).astype(np.int64)
    hmask = np.asarray(inputs["head_mentions_indices_mask"]).astype(np.int64)
    tidx = np.asarray(inputs["tail_mentions_indices"]).astype(np.int64)
    tmask = np.asarray(inputs["tail_mentions_indices_mask"]).astype(np.int64)
    hti = np.asarray(inputs["ht_comb_indices"]).astype(np.int64)
    htm = np.asarray(inputs["ht_comb_mask"]).astype(np.int64)
    w1h = np.ascontiguousarray(np.asarray(inputs["W1h"], np.float32))
    w2h = np.ascontiguousarray(np.asarray(inputs["W2h"], np.float32))
    w1t = np.ascontiguousarray(np.asarray(inputs["W1t"], np.float32))
    w2t = np.ascontiguousarray(np.asarray(inputs["W2t"], np.float32))
    bili_W = np.asarray(inputs["bili_W"], np.float32)
    bili_b = np.asarray(inputs["bili_b"], np.float32)

    import ml_dtypes

    wde = np.ascontiguousarray(
        bili_W.transpose(1, 0, 2).reshape(FF, RE).astype(ml_dtypes.bfloat16)
    )
    spat = np.zeros((128, 4), np.float32)
    for g in range(4):
        spat[32 * g:32 * g + T, g] = 1.0
    bvec = np.broadcast_to(bili_b[None, :], (4, R)).copy()

    s_ = spans[..., 0]                       # (N, M)
    e_ = spans[..., 1]

    in_maps = []
    for c in range(NCORES):
        ns = slice(c * NL, (c + 1) * NL)
        # gather indices + span-average patterns
        gidx16 = np.zeros((16, NL * 16), np.int16)
        gpat = np.zeros((128, NL, 2, M), np.float32)
        sc_, ec_ = s_[ns], e_[ns]
        for n in range(NL):
            for slot in range(32):
                for w in range(8):
                    i = n * 256 + slot * 8 + w
                    if slot < M:
                        row = n * L + int(sc_[n, slot]) + w
                        ln = int(ec_[n, slot]) - int(sc_[n, slot]) + 1
                        if w < ln:
                            gpat[(slot % 16) * 8 + w, n, slot // 16, slot] = 1.0 / ln
                    else:
                        row = 0
                    gidx16[i % 16, i // 16] = row
        gidxa = np.tile(gidx16, (8, 1))      # replicate to 128 partitions
        gbh = np.zeros((SP, NH), np.float32)
        gbt = np.zeros((SP, NH), np.float32)
        for n in range(NL):
            # padded span row for (n, m): 128*(n//4) + 32*(n%4) + m
            r0 = 128 * (n // 4) + 32 * (n % 4)
            for i in range(H):
                gbh[r0 + hidx[ns][n, i], n * H + i] = float(hmask[ns][n, i])
                gbt[r0 + tidx[ns][n, i], n * T + i] = float(tmask[ns][n, i])
        cnt = np.zeros((4, 128, NI), np.float32)
        for n in range(NL):
            t2, g = divmod(n, 4)
            for p in range(HT):
                if htm[ns][n, p]:
                    i, j = hti[ns][n, p, 0], hti[ns][n, p, 1]
                    # partition 32*g+j, free col r*10+i for every r
                    cnt[t2, 32 * g + j, i::10] += 1.0
        in_maps.append(
            dict(
                sent=np.ascontiguousarray(sent_f[ns].reshape(NL * L, D)),
                gidx=gidxa, gpat=gpat,
                w1h=w1h, w2h=w2h, w1t=w1t, w2t=w2t,
                gbh=gbh, gbt=gbt, wde=wde,
                cnt=cnt, spat=spat, bvec=bvec,
            )
        )
    return in_maps


def kernel(**inputs) -> np.ndarray:
    from concourse.bass_utils import run_bass_kernel_spmd

    cache = _build()
    in_maps = _host_prep(inputs)
    res = run_bass_kernel_spmd(cache["nc"], in_maps, list(range(NCORES)))
    out = np.concatenate([res.results[c]["out"] for c in range(NCORES)], axis=0)
    return out.astype(np.float32)



# revision 27
# speedup vs baseline: 2.0302x; 2.0302x over previous
"""Trainium2 Bass kernel for nn_BiAffineForward (bilinear relation scorer).

Data-parallel over N=128 across 8 NeuronCores (16 batches/core). All heavy
matmuls run in bf16 (fp32 runs at half PE rate); index structures become
host-built gather lists / one-hot matmul patterns.

Per-core pipeline:
  P1  dedup'd span-row gather (bf16, trailing-negative-trimmed) + span-mean
      via a 128-col block-packed stationary -> spans_A [128(strip), 4, 768]
  T1  SBUF-source transpose dma_gather -> spansT [d, 512 slots]  (off PE)
  P2  two-layer MLP in bf16 on packed 384 span cols
  P3  mention gather+mask via one-hot matmul -> hg/tg [140, 160] bf16
  P4  bilinear stage 1 per r (halves of 49/48): psA [e<128, (n,i)] and
      psB (e-rem, col-tiled 4 r's/bank -> striped hwrem layout)
  P5  stage 2: tails col-tiled 4 batches/bank + 16-tile e-remainder pass,
      fused evac-add -> L [128(g,j), (r,i)]
  P6  count-trick masked logsumexp (exp on ACT, cnt-mul on GpSimd,
      reduce on DVE, partition-sum via tiny fp32 matmul)

logsumexp uses no max-subtraction: logits are O(1e-2) by construction and
masked pairs contribute exactly 0 (count C=0), matching exp(-1e10)==0.
"""

import sys

import numpy as np

sys.path.insert(0, "/opt/trn_rl_repo")

N, L, D = 128, 512, 768
M, H, T, HT = 24, 10, 10, 100
FF, R = 140, 97
NCORES = 8
NL = N // NCORES          # 16 batches per core
NH = NL * H               # 160 (n,i) columns per core
RE = R * FF               # 13580
NI = R * H                # 970
HALVES = ((0, 49), (49, 48))
GQ = 4                    # gather groups (4 batches each)
GIDX_N = 768              # static idx slots per gather group

_cache = {}


def _build(maxcnt):
    key = tuple(int(x) for x in maxcnt)
    if key in _cache:
        return _cache[key]
    from contextlib import ExitStack

    import concourse.bacc as bacc
    import concourse.bass as bass
    import concourse.mybir as mybir
    import concourse.tile as tile

    f32 = mybir.dt.float32
    bf16 = mybir.dt.bfloat16
    i16 = mybir.dt.int16
    AF = mybir.ActivationFunctionType

    nc = bacc.Bacc(
        "TRN2",
        target_bir_lowering=False,
        debug=False,
        num_devices=NCORES,
        num_swdge_queues=4,
    )

    def din(name, shape, dt=bf16):
        return nc.dram_tensor(name, list(shape), dt, kind="ExternalInput").ap()

    sent = din("sent", (NL * L, D))
    gidx = din("gidx", (128, GQ * (GIDX_N // 16)), i16)
    tidx = din("tidx", (128, 32), i16)
    gpat = din("gpat", (128, GQ, 6, 128))
    w1h = din("w1h", (D, FF))
    w2h = din("w2h", (FF, FF))
    w1t = din("w1t", (D, FF))
    w2t = din("w2t", (FF, FF))
    gbh = din("gbh", (384, NH))
    gbt = din("gbt", (384, NH))
    wde = din("wde", (FF, R * 128))      # main e<128 cols only, r-major
    wrem = din("wrem", (FF, 25 * 128))   # e-remainder packed at 32-strips
    cntd = din("cnt", (4, 128, NI), f32)
    spat = din("spat", (128, 4), f32)
    bvec = din("bvec", (4, R), f32)
    outp = nc.dram_tensor("out", [NL, R], f32, kind="ExternalOutput").ap()
    import os
    dbg = os.environ.get("KDBG", "0") == "1"
    if dbg:
        d_hwt = nc.dram_tensor("d_hwt", [128, NL * 490], mybir.dt.bfloat16,
                               kind="ExternalOutput").ap()
        d_hwr = nc.dram_tensor("d_hwr", [128, 13 * NH], mybir.dt.bfloat16,
                               kind="ExternalOutput").ap()
        d_lt = nc.dram_tensor("d_lt", [128, 4 * NI], f32,
                              kind="ExternalOutput").ap()
        d_hg = nc.dram_tensor("d_hg", [128, 4 * NH], mybir.dt.bfloat16,
                              kind="ExternalOutput").ap()

    with tile.TileContext(nc) as tc, ExitStack() as ctx:
        const = ctx.enter_context(tc.tile_pool(name="const", bufs=1))
        persist = ctx.enter_context(tc.tile_pool(name="persist", bufs=1))

        # ---- constants into SBUF
        w1h_sb = const.tile([128, 6, FF], bf16)
        nc.sync.dma_start(w1h_sb[:], w1h.rearrange("(c p) f -> p c f", p=128))
        w1t_sb = const.tile([128, 6, FF], bf16)
        nc.sync.dma_start(w1t_sb[:], w1t.rearrange("(c p) f -> p c f", p=128))
        w2h_sb = const.tile([128, 2, FF], bf16)
        nc.sync.dma_start(w2h_sb[:, 0, :], w2h[0:128, :])
        nc.sync.dma_start(w2h_sb[0:12, 1, :], w2h[128:FF, :])
        w2t_sb = const.tile([128, 2, FF], bf16)
        nc.sync.dma_start(w2t_sb[:, 0, :], w2t[0:128, :])
        nc.sync.dma_start(w2t_sb[0:12, 1, :], w2t[128:FF, :])
        gbh_sb = const.tile([128, 3, NH], bf16)
        nc.sync.dma_start(gbh_sb[:], gbh.rearrange("(c p) g -> p c g", p=128))
        gbt_sb = const.tile([128, 3, NH], bf16)
        nc.sync.dma_start(gbt_sb[:], gbt.rearrange("(c p) g -> p c g", p=128))
        cnt_sb = const.tile([128, 4, NI], f32)
        nc.sync.dma_start(cnt_sb[:], cntd.rearrange("t p f -> p t f"))
        spat_sb = const.tile([128, 4], f32)
        nc.sync.dma_start(spat_sb[:], spat[:, :])
        bvec_sb = const.tile([4, R], f32)
        nc.sync.dma_start(bvec_sb[:], bvec[:, :])
        gidx_sb = const.tile([128, GQ * (GIDX_N // 16)], i16)
        nc.sync.dma_start(gidx_sb[:], gidx[:, :])
        tidx_sb = const.tile([128, 32], i16)
        nc.sync.dma_start(tidx_sb[:], tidx[:, :])
        gpat_sb = const.tile([128, GQ, 6, 128], bf16)
        nc.sync.dma_start(gpat_sb[:], gpat[:, :, :, :])

        # ---- persistent tensors
        wt0 = persist.tile([128, R * 128], bf16)   # wde rows d<128
        nc.sync.dma_start(wt0[:], wde[0:128, :])
        wt1 = persist.tile([12, R * 128], bf16)    # wde rows d=128..139
        nc.sync.dma_start(wt1[:], wde[128:FF, :])
        wr0 = persist.tile([128, 25 * 128], bf16)
        nc.sync.dma_start(wr0[:], wrem[0:128, :])
        wr1 = persist.tile([12, 25 * 128], bf16)
        nc.sync.dma_start(wr1[:], wrem[128:FF, :])
        hg0 = persist.tile([128, NH], bf16)
        hg1 = persist.tile([12, NH], bf16)
        tg0 = persist.tile([128, NH], bf16)
        tg1 = persist.tile([12, NH], bf16)
        spans_A = persist.tile([128, GQ, D], bf16)
        spansT = persist.tile([128, 6, 512], bf16)
        lts = [persist.tile([128, NI], f32, name=f"lt{t}") for t in range(4)]
        for lt in lts:  # unwritten partition rows must exp() to finite values
            nc.gpsimd.memset(lt[:], 0.0)

        # =========================== Phase A ===========================
        with tc.tile_pool(name="pa_str", bufs=2) as pstr, \
             tc.tile_pool(name="pa_ps", bufs=1, space="PSUM") as pps, \
             tc.tile_pool(name="pa_ps2", bufs=1, space="PSUM") as pps2, \
             tc.tile_pool(name="pa_ps3", bufs=2, space="PSUM") as pps3, \
             tc.tile_pool(name="pa_ps4", bufs=1, space="PSUM") as pps4, \
             tc.tile_pool(name="pa_sb", bufs=1) as pa:

            for q in range(GQ):
                ncc = (int(maxcnt[q]) + 127) // 128  # live gather chunks
                gt = pstr.tile([128, 6, D], bf16, tag="gt")
                nc.gpsimd.dma_gather(
                    out_ap=gt[:, :, :],
                    in_ap=sent[:, :],
                    idxs_ap=gidx_sb[:, q * 48:(q + 1) * 48],
                    num_idxs=GIDX_N,
                    num_idxs_reg=ncc * 128,
                    elem_size=D,
                    queue_num=q % 4,
                )
                ps1 = pps.tile([128, 2, 512], f32, tag="ps1")
                for h in range(2):
                    for gc in range(ncc):
                        nc.tensor.matmul(
                            ps1[:, h, 0:384],
                            lhsT=gpat_sb[:, q, gc, :],
                            rhs=gt[:, gc, h * 384:(h + 1) * 384],
                            start=(gc == 0),
                            stop=(gc == ncc - 1),
                        )
                nc.vector.tensor_copy(spans_A[:, q, 0:384], ps1[:, 0, 0:384])
                nc.scalar.copy(spans_A[:, q, 384:768], ps1[:, 1, 0:384])

            # T1: SBUF-source transpose gather -> spansT [d, slot]
            nc.gpsimd.dma_gather(
                out_ap=spansT[:, :, :],
                in_ap=spans_A[:, :, :],
                idxs_ap=tidx_sb[:, :],
                num_idxs=512,
                num_idxs_reg=512,
                elem_size=D,
                transpose=True,
                queue_num=0,
                sbuf_tokens_per_rank=128,
                sbuf_free_dim_per_rank=D * 2,
            )

            # P2 + P3 per side (packed 384 span cols)
            rhs_packed = [
                spansT[:, kc, :]
                .rearrange("p (x m) -> p x m", m=32)[:, :, 0:24]
                for kc in range(6)
            ]
            for w1sb, w2sb, gbsb, g0, g1 in (
                (w1h_sb, w2h_sb, gbh_sb, hg0, hg1),
                (w1t_sb, w2t_sb, gbt_sb, tg0, tg1),
            ):
                ps2a = pps2.tile([128, 512], f32, tag="ps2a")
                ps2b = pps2.tile([12, 512], f32, tag="ps2b")
                for kc in range(6):
                    nc.tensor.matmul(
                        ps2a[:, 0:384], lhsT=w1sb[:, kc, 0:128],
                        rhs=rhs_packed[kc], start=(kc == 0), stop=(kc == 5),
                    )
                for kc in range(6):
                    nc.tensor.matmul(
                        ps2b[:, 0:384], lhsT=w1sb[:, kc, 128:FF],
                        rhs=rhs_packed[kc], start=(kc == 0), stop=(kc == 5),
                    )
                a0 = pa.tile([128, 384], bf16, tag="a0")
                a1 = pa.tile([12, 384], bf16, tag="a1")
                nc.scalar.activation(a0[:], ps2a[:, 0:384], AF.Relu)
                nc.vector.tensor_relu(a1[:], ps2b[:, 0:384])
                b2 = pa.tile([128, 3, FF], bf16, tag="b2")
                for sc in range(3):
                    ps2c = pps3.tile([128, FF], f32, tag="ps2c")
                    sl = slice(sc * 128, (sc + 1) * 128)
                    nc.tensor.matmul(
                        ps2c[:], lhsT=a0[:, sl], rhs=w2sb[:, 0, :],
                        start=True, stop=False,
                    )
                    nc.tensor.matmul(
                        ps2c[:], lhsT=a1[:, sl], rhs=w2sb[0:12, 1, :],
                        start=False, stop=True,
                    )
                    if sc % 2 == 0:
                        nc.vector.tensor_copy(b2[:, sc, :], ps2c[:])
                    else:
                        nc.scalar.copy(b2[:, sc, :], ps2c[:])
                ps3 = pps4.tile([128, NH], f32, tag="ps3")
                ps3r = pps4.tile([12, NH], f32, tag="ps3r")
                for sc in range(3):
                    nc.tensor.matmul(
                        ps3[:], lhsT=b2[:, sc, 0:128], rhs=gbsb[:, sc, :],
                        start=(sc == 0), stop=(sc == 2),
                    )
                for sc in range(3):
                    nc.tensor.matmul(
                        ps3r[:], lhsT=b2[:, sc, 128:FF], rhs=gbsb[:, sc, :],
                        start=(sc == 0), stop=(sc == 2),
                    )
                nc.vector.tensor_copy(g0[:], ps3[:])
                nc.scalar.copy(g1[:], ps3r[:])

        # =========================== Phase B ===========================
        with tc.tile_pool(name="pb_sb", bufs=1) as pb, \
             tc.tile_pool(name="p6_sb", bufs=2) as p6:
            for hi, (rh0, rhw) in enumerate(HALVES):
                FR = rhw * H
                K4 = (rhw + 3) // 4
                K4m = rhw // 4  # full 4-r groups
                g0off = 0 if hi == 0 else 13  # wrem group offset
                with tc.tile_pool(name="pbA", bufs=4, space="PSUM") as bpsA, \
                     tc.tile_pool(name="pbB", bufs=2, space="PSUM") as bpsB:
                    hwt = pb.tile([128, NL * FR], bf16, tag="hwt")
                    hwv = hwt.rearrange("p (n r i) -> p n r i", n=NL, r=rhw)
                    hwrem = pb.tile([128, K4 * NH], bf16, tag="hwrem")
                    for k in range(K4):
                        njj = min(4, rhw - 4 * k)
                        pA = [
                            bpsA.tile([128, 2, NH], f32, tag="pA",
                                      name=f"pA{rh0}_{k}_{h2}")
                            for h2 in range((njj + 1) // 2)
                        ]
                        pB = bpsB.tile([128, NH], f32, tag="pB")
                        for jj in range(njj):
                            rr = rh0 + 4 * k + jj
                            nc.tensor.matmul(
                                pA[jj // 2][:, jj % 2, :],
                                lhsT=wt0[:, rr * 128:rr * 128 + 128],
                                rhs=hg0[:],
                                start=(jj % 2 == 0), stop=False,
                            )
                        for jj in range(njj):
                            rr = rh0 + 4 * k + jj
                            last = (jj % 2 == 1) or (jj == njj - 1)
                            nc.tensor.matmul(
                                pA[jj // 2][:, jj % 2, :],
                                lhsT=wt1[:, rr * 128:rr * 128 + 128],
                                rhs=hg1[:],
                                start=False, stop=last,
                            )
                        # e-remainder: 4 r's packed at 32-col strips
                        gi = g0off + k
                        nc.tensor.matmul(
                            pB[:, :],
                            lhsT=wr0[:, gi * 128:(gi + 1) * 128],
                            rhs=hg0[:], start=True, stop=False,
                        )
                        nc.tensor.matmul(
                            pB[:, :],
                            lhsT=wr1[:, gi * 128:(gi + 1) * 128],
                            rhs=hg1[:], start=False, stop=True,
                        )
                        # evacuate: psA -> hwt [p, n, r, i]; psB -> hwrem
                        for half2 in range((njj + 1) // 2):
                            w = min(2, njj - 2 * half2)
                            r0 = 4 * k + 2 * half2
                            nc.vector.tensor_copy(
                                hwv[:, :, r0:r0 + w, :],
                                pA[half2][:, 0:w, :].rearrange(
                                    "p r (n i) -> p n r i", n=NL
                                ),
                            )
                        nc.scalar.copy(hwrem[:, k * NH:k * NH + NH], pB[:, :])

                if dbg and rh0 == 0:
                    nc.sync.dma_start(d_hwt[:, :], hwt[:, :])
                    nc.sync.dma_start(d_hwr[:, :], hwrem[:, :])
                # ---- flatten hwrem strips -> [12, (rho, n, i)] via DMA
                hwf = pb.tile([12, FR * NL], bf16, tag="hwf")
                hfm = hwf[:, 0:K4m * 4 * NH].rearrange(
                    "p (k j x) -> p k j x", j=4, x=NH
                )
                for j in range(4):
                    nc.sync.dma_start(
                        hfm[:, :, j, :],
                        hwrem[32 * j:32 * j + 12, 0:K4m * NH].rearrange(
                            "p (k x) -> p k x", x=NH
                        ),
                    )
                if rhw % 4:
                    nc.sync.dma_start(
                        hwf[:, K4m * 4 * NH:(K4m * 4 + 1) * NH],
                        hwrem[0:12, K4m * NH:(K4m + 1) * NH],
                    )
                hwfv = hwf.rearrange("p (r n i) -> p r n i", r=rhw, n=NL)
                # ---- P5 for this half: per-batch, main + remainder
                with tc.tile_pool(name="p5m", bufs=4, space="PSUM") as p5m:
                    for n in range(NL):
                        ps5 = p5m.tile([10, 512], f32, tag="ps5")
                        nc.tensor.matmul(
                            ps5[:, 0:FR],
                            lhsT=tg0[:, n * H:(n + 1) * H],
                            rhs=hwt[:, n * FR:(n + 1) * FR],
                            start=True, stop=False,
                        )
                        nc.tensor.matmul(
                            ps5[:, 0:FR],
                            lhsT=tg1[:, n * H:(n + 1) * H],
                            rhs=hwfv[:, :, n, :],
                            start=False, stop=True,
                        )
                        t, g = divmod(n, 4)
                        dst = lts[t][32 * g:32 * g + 10,
                                     rh0 * H:(rh0 + rhw) * H]
                        if n % 2 == 0:
                            nc.vector.tensor_copy(dst, ps5[:, 0:FR])
                        else:
                            nc.scalar.copy(dst, ps5[:, 0:FR])

            if dbg:
                nc.sync.dma_start(d_hg[:, 0:NH], hg0[:, :])
                nc.sync.dma_start(d_hg[0:12, NH:2 * NH], hg1[:, :])
                nc.sync.dma_start(d_hg[:, 2 * NH:3 * NH], tg0[:, :])
                nc.sync.dma_start(d_hg[0:12, 3 * NH:4 * NH], tg1[:, :])
                for t in range(4):
                    nc.sync.dma_start(d_lt[:, t * NI:(t + 1) * NI], lts[t][:, :])
            # ---- P6: count-trick masked logsumexp
            with tc.tile_pool(name="p6ps", bufs=2, space="PSUM") as p6ps:
                for t in range(4):
                    lt = lts[t]
                    et = p6.tile([128, NI], f32, tag="et")
                    nc.scalar.activation(et[:], lt[:], AF.Exp)
                    mt = p6.tile([128, NI], f32, tag="mt")
                    nc.gpsimd.tensor_mul(mt[:], et[:], cnt_sb[:, t, :])
                    s1 = p6.tile([128, R], f32, tag="s1")
                    nc.vector.tensor_reduce(
                        s1[:],
                        mt.rearrange("p (r i) -> p r i", r=R),
                        axis=mybir.AxisListType.X,
                        op=mybir.AluOpType.add,
                    )
                    ps6 = p6ps.tile([4, R], f32, tag="ps6")
                    nc.tensor.matmul(
                        ps6[:], lhsT=spat_sb[:], rhs=s1[:], start=True, stop=True
                    )
                    lg = p6.tile([4, R], f32, tag="lg")
                    nc.scalar.activation(lg[:], ps6[:], AF.Ln)
                    res = p6.tile([4, R], f32, tag="res")
                    nc.vector.tensor_add(res[:], lg[:], bvec_sb[:])
                    nc.sync.dma_start(outp[t * 4:(t + 1) * 4, :], res[:])

    nc.compile()
    _cache[key] = nc
    return nc


def _host_prep(inputs):
    """Shard + build index-derived matrices. Returns (in_maps, maxcnt)."""
    import ml_dtypes

    nbf = ml_dtypes.bfloat16
    sent_f = np.asarray(inputs["sentence_repr"], np.float32)
    spans = np.asarray(inputs["entity_span_indices"]).astype(np.int64)
    hidx = np.asarray(inputs["head_mentions_indices"]).astype(np.int64)
    hmask = np.asarray(inputs["head_mentions_indices_mask"]).astype(np.int64)
    tidx_i = np.asarray(inputs["tail_mentions_indices"]).astype(np.int64)
    tmask = np.asarray(inputs["tail_mentions_indices_mask"]).astype(np.int64)
    hti = np.asarray(inputs["ht_comb_indices"]).astype(np.int64)
    htm = np.asarray(inputs["ht_comb_mask"]).astype(np.int64)

    s_, e_ = spans[..., 0], spans[..., 1]

    # --- per (core, group) dedup'd row lists + membership weights
    rows_cq = [[None] * GQ for _ in range(NCORES)]
    memb_cq = [[None] * GQ for _ in range(NCORES)]
    counts = np.zeros((NCORES, GQ), np.int64)
    for c in range(NCORES):
        for q in range(GQ):
            rows, memb = [], []
            for g in range(4):
                nn = c * NL + 4 * q + g
                rset = {}
                for m in range(M):
                    s0, e0 = int(s_[nn, m]), int(e_[nn, m])
                    w = 1.0 / (e0 - s0 + 1)
                    for r in range(s0, e0 + 1):
                        rset.setdefault(r, []).append((32 * g + m, w))
                for r in sorted(rset):
                    rows.append((4 * q + g) * L + r)
                    memb.append(rset[r])
            counts[c, q] = len(rows)
            rows_cq[c][q], memb_cq[c][q] = rows, memb
    maxcnt = counts.max(axis=0)

    spat = np.zeros((128, 4), np.float32)
    for g in range(4):
        spat[32 * g:32 * g + T, g] = 1.0
    bvec = np.broadcast_to(
        np.asarray(inputs["bili_b"], np.float32)[None, :], (4, R)
    ).copy()
    bili_W = np.asarray(inputs["bili_W"], np.float32)
    # main (e<128) cols, r-major: wde[d, r*128+e] = W[r, d, e]
    wde = np.ascontiguousarray(
        bili_W[:, :, 0:128].transpose(1, 0, 2).reshape(FF, R * 128)
    ).astype(nbf)
    # e-remainder packed: 4 r's per 128-col group at 32-col strips
    wrem = np.zeros((FF, 25 * 128), np.float32)
    for hi2, (rh0, rhw) in enumerate(HALVES):
        g0off = 0 if hi2 == 0 else 13
        for k in range((rhw + 3) // 4):
            for jj in range(min(4, rhw - 4 * k)):
                r = rh0 + 4 * k + jj
                wrem[:, (g0off + k) * 128 + 32 * jj:
                     (g0off + k) * 128 + 32 * jj + 12] = bili_W[r, :, 128:FF]
    wrem = wrem.astype(nbf)
    tidx16 = -np.ones((16, 32), np.int16)
    for i in range(512):
        tidx16[i % 16, i // 16] = i
    tidxa = np.tile(tidx16, (8, 1))
    shared = dict(
        w1h=np.asarray(inputs["W1h"], np.float32).astype(nbf),
        w2h=np.asarray(inputs["W2h"], np.float32).astype(nbf),
        w1t=np.asarray(inputs["W1t"], np.float32).astype(nbf),
        w2t=np.asarray(inputs["W2t"], np.float32).astype(nbf),
        wde=wde, wrem=wrem, spat=spat, bvec=bvec, tidx=tidxa,
    )

    in_maps = []
    for c in range(NCORES):
        ns = slice(c * NL, (c + 1) * NL)
        gidx16 = -np.ones((16, GQ * 48), np.int16)
        gpat = np.zeros((128, GQ, 6, 128), np.float32)
        for q in range(GQ):
            rows = list(rows_cq[c][q])
            memb = list(memb_cq[c][q])
            # pad to a 128 multiple with dup row 0 (zero weights) so every
            # live gather chunk is fully transferred (no stale-SBUF NaNs)
            cpad = ((int(maxcnt[q]) + 127) // 128) * 128
            while len(rows) < cpad:
                rows.append(0)
                memb.append([])
            for i, (r, mb) in enumerate(zip(rows, memb)):
                gidx16[i % 16, q * 48 + i // 16] = r
                for col, w in mb:
                    gpat[i % 128, q, i // 128, col] += w
        gidx = np.tile(gidx16, (8, 1))

        gbh = np.zeros((384, NH), np.float32)
        gbt = np.zeros((384, NH), np.float32)
        for n in range(NL):
            for i in range(H):
                gbh[24 * n + hidx[ns][n, i], n * H + i] = float(hmask[ns][n, i])
                gbt[24 * n + tidx_i[ns][n, i], n * T + i] = float(tmask[ns][n, i])
        cnt = np.zeros((4, 128, NI), np.float32)
        for n in range(NL):
            t2, g = divmod(n, 4)
            for p in range(HT):
                if htm[ns][n, p]:
                    i, j = hti[ns][n, p, 0], hti[ns][n, p, 1]
                    cnt[t2, 32 * g + j, i::10] += 1.0
        im = dict(
            sent=np.ascontiguousarray(sent_f[ns].reshape(NL * L, D)).astype(nbf),
            gidx=gidx, gpat=gpat.astype(nbf),
            gbh=gbh.astype(nbf), gbt=gbt.astype(nbf), cnt=cnt,
        )
        im.update(shared)
        in_maps.append(im)
    return in_maps, maxcnt


def kernel(**inputs) -> np.ndarray:
    from concourse.bass_utils import run_bass_kernel_spmd

    in_maps, maxcnt = _host_prep(inputs)
    nc = _build(maxcnt)
    res = run_bass_kernel_spmd(nc, in_maps, list(range(NCORES)))
    out = np.concatenate([res.results[c]["out"] for c in range(NCORES)], axis=0)
    return out.astype(np.float32)
